# revision 19
# baseline (speedup 1.0000x reference)
"""DeformConv2dBlock (offset-conv -> deformable 3x3 conv -> train-mode BN -> ReLU)
as a Bass/Tile SPMD kernel on 8 TRN2 NeuronCores.

Sharding: data-parallel over (batch n, image half): core s handles
n = s//2, rows h0 = (s%2)*48 .. h0+48.  Params replicated.  BN batch stats
get an 8-core AllReduce for exact training-mode parity.

v3 pipeline (per core) -- replication-matmul-free:
  1. Host pre-builds XT4, a position-major QUAD-token buffer with
     corner-per-quadrant element order: token q element (c_hi, d, c_lo)
     = slab[32*c_hi + c_lo, q + delta_d], delta = (0, 1, WP, WP+1).
     After the transposing gather, corner d's data lands on partition
     quadrant d: g4[32*d + c_lo, c_hi, tok].
  2. PE: offset conv (9 shifted matmuls, bf16).  DVE/ACT: sampling
     positions, frac via f32 mod, bilinear corner weight planes wq
     (bf16, quadrant d rows 32d+k = corner-d weight of tap k), int16
     anchors.  Everything runs in gather-token order (host permutes the
     base grid), so no strided engine reads anywhere in the main loop.
  3. stream_shuffle (DVE 32x32 reorder, mask=[k]*32) broadcasts tap k's
     corner weights across each quadrant: ONE op replaces the per-corner
     K=9 replication matmuls of v2.
  4. Q7: ONE dma_gather per (half, tap, 768-chunk), rotated over 4 SWDGE
     queues so transfers overlap descriptor-gen.
  5. DVE: ag4 = g4 * Wr4 (all-bf16, 2x mode).  PE: deform conv via
     4x-row-replicated weights wdef4[k, c_hi]: psum[o,t] += sum_{32d+c_lo}
     wdef[32c_hi+c_lo, o] * w_d(t) * x[32c_hi+c_lo, anchor_t + delta_d],
     accumulated over (k, c_hi) -- 36 matmul-pairs per 768-token tile.
  6. ACT: per-channel sum/sumsq, AllReduce over 8 cores, in-place BN+ReLU
     on bf16 convout, host unpermutes token->position order.
"""

import numpy as np
import ml_dtypes

# ---------------------------------------------------------------- constants
N, C, O, H, W = 4, 128, 128, 96, 96
KT = 9                      # 3x3 taps
PAD = 4
HSH = H // 2                # 48 rows per shard
HP = HSH + 2 * PAD          # 56 slab rows
WP = W + 2 * PAD            # 104 slab cols
SLAB = HP * WP              # 5824
XLEN = 6160                 # padded slab row length (>= SLAB + WP + 2, /16)
NTOK = 6144                 # quad tokens in XT4 (48 ranks * 128)
NPOS = HSH * W              # 4608 output positions per core
HALFP = NPOS // 2           # 2304
GCH = 768                   # gather chunk (descriptor-ring limit < 1024)
NCH = HALFP // GCH          # 3
TILE = GCH                  # psum col tile
NCORES = 8
BN_EPS = 1e-5
NELEM = N * H * W           # BN normalizer 36864

_prog_cache = {}


def _token_perm():
    """token t (within a 2304-position half) -> position p'.

    The device stores position p' at token t = 144*(p'%16) + p'//16 (the
    gather's 16-partition wrap); the inverse is p' = 16*(t%144) + t//144.
    """
    t = np.arange(HALFP)
    return 16 * (t % 144) + t // 144


def _build_program(phase=4):
    import concourse.bass as bass
    import concourse.bacc as bacc
    import concourse.tile as tile
    import concourse.mybir as mybir

    f32 = mybir.dt.float32
    bf16 = mybir.dt.bfloat16
    i16 = mybir.dt.int16
    AF = mybir.ActivationFunctionType
    ALU = mybir.AluOpType

    nc = bacc.Bacc("TRN2", target_bir_lowering=False, num_devices=NCORES,
                   num_swdge_queues=4)

    xb_d = nc.declare_dram_parameter("xb", [C, XLEN], bf16, isOutput=False)
    xt4_d = nc.declare_dram_parameter("xt4", [C, 4 * NTOK], bf16, isOutput=False)
    tb_d = nc.declare_dram_parameter("tb", [C, 2, NPOS], f32, isOutput=False)
    woff_d = nc.declare_dram_parameter("woff", [C, KT, 128], bf16, isOutput=False)
    wdef_d = nc.declare_dram_parameter("wdef", [C, KT * 4 * O], bf16,
                                       isOutput=False)
    bn_d = nc.declare_dram_parameter("bn", [O, 2], f32, isOutput=False)
    y_d = nc.declare_dram_parameter("y", [O, NPOS], bf16, isOutput=True)

    ccin = nc.dram_tensor("ccin", [O, 2], f32)
    ccout = nc.dram_tensor("ccout", [O, 2], f32)
    # anchor-index staging in DRAM (straight copy; reload does the 16-wrap)
    stgi = nc.dram_tensor("stgi", [2, KT, HALFP], i16)

    def vap(t, off_el, dims):
        """Raw AP view of tile t at element offset off_el with free dims."""
        a = t[:]
        return bass.AP(a.tensor, a.offset + off_el, [a.ap[0]] + dims)

    from concourse.tile import add_dep_helper

    with tile.TileContext(nc) as tc:
        import contextlib
        est = contextlib.ExitStack()
        with est:
            const = est.enter_context(tc.tile_pool(name="const", bufs=1))

            woff = const.tile([C, KT, 128], bf16)
            nc.sync.dma_start(out=woff, in_=woff_d[:, :, :])
            wdef4 = const.tile([C, KT, 4, O], bf16)
            nc.sync.dma_start(out=wdef4[:], in_=wdef_d[:, :])
            bn = const.tile([O, 2], f32)
            nc.sync.dma_start(out=bn, in_=bn_d[:, :])

            XT4 = const.tile([C, 4 * NTOK], bf16)      # quad-token buffer
            idx_w = const.tile([C, 2, KT, 144], i16)   # 16-wrapped anchor idx
            # corner weight planes, token order, bf16.  Row 32d + k holds
            # corner d's weight for tap k (replica rows 32d+9..32d+31 unused).
            wq = const.tile([C, NPOS], bf16)
            # offsets in token order: row 32r + j = tap j (replica r),
            # free dims (yx, token)
            off_h = const.tile([C, 2, HALFP], bf16)
            nc.vector.memset(off_h, 0.0)

            convout = const.tile([O, NPOS], bf16)          # pre-BN conv out
            sums = const.tile([O, 2 * NCH], f32)
            sqsums = const.tile([O, 2 * NCH], f32)

            xbp = est.enter_context(tc.tile_pool(name="xbp", bufs=1))
            psA = est.enter_context(
                tc.tile_pool(name="psA", bufs=2, space="PSUM"))
            plp = est.enter_context(tc.tile_pool(name="plp", bufs=1))
            wrp = est.enter_context(tc.tile_pool(name="wrp", bufs=2))
            gp = est.enter_context(tc.tile_pool(name="gp", bufs=2))
            psD = est.enter_context(
                tc.tile_pool(name="psD", bufs=3, space="PSUM"))

            xb = xbp.tile([C, XLEN], bf16)
            nc.sync.dma_start(out=xb, in_=xb_d[:, :])
            nc.sync.dma_start(out=XT4, in_=xt4_d[:, :])

            gctr = 0
            for h3 in range(2):
                # ---------------- offset conv (position order in PSUM) -----
                for t in range(6):
                    T = h3 * 6 + t
                    po = psA.tile([C, 384], f32, tag="poff", name=f"po{T}")
                    for k in range(KT):
                        ky, kx = k // 3, k % 3
                        rhs = vap(xb, (4 * T + ky + 3) * WP + kx + 3,
                                  [[WP, 4], [1, W]])
                        nc.tensor.matmul(po[:], woff[:, k, :], rhs,
                                         start=(k == 0), stop=(k == KT - 1))
                    # ostg column order col' = 24b + a' (position q = 16a'+b)
                    # so the token-order redistribution DMA reads it flat
                    ostg = xbp.tile([C, 384], bf16, tag="ostg",
                                    name=f"ostg{T}")
                    pa = po[:]
                    sa = ostg[:]
                    nc.scalar.activation(
                        out=bass.AP(sa.tensor, sa.offset,
                                    [sa.ap[0], [1, 24], [24, 16]]),
                        in_=bass.AP(pa.tensor, pa.offset,
                                    [pa.ap[0], [16, 24], [1, 16]]),
                        func=AF.Copy)
                    oa = off_h[:]
                    ostride = oa.ap[0][0]
                    sstride = sa.ap[0][0]
                    for r in range(4):
                        for yx in range(2):
                            dst = bass.AP(
                                oa.tensor,
                                oa.offset + 32 * r * ostride
                                + yx * HALFP + 24 * t,
                                [[ostride, 9], [144, 16], [1, 24]])
                            src = bass.AP(
                                sa.tensor,
                                sa.offset + (32 * r + 9 * yx) * sstride,
                                [[sstride, 9], [1, 384]])
                            nc.sync.dma_start(out=dst, in_=src)

                # ---------------- positions (token order) ------------------
                pp = plp.tile([C, 2, HALFP], f32, tag="pp", name=f"pp{h3}")
                nc.sync.dma_start(
                    out=pp, in_=tb_d[:, :, h3 * HALFP: (h3 + 1) * HALFP])
                nc.vector.tensor_tensor(out=pp, in0=pp, in1=off_h, op=ALU.add)
                # clamp sampling coords into the zero-padded slab
                nc.vector.tensor_scalar(out=pp[:, 0, :], in0=pp[:, 0, :],
                                        scalar1=0.01,
                                        scalar2=float(HP - 1.1),
                                        op0=ALU.max, op1=ALU.min)
                nc.vector.tensor_scalar(out=pp[:, 1, :], in0=pp[:, 1, :],
                                        scalar1=0.01,
                                        scalar2=float(WP - 1.1),
                                        op0=ALU.max, op1=ALU.min)
                # floor via int cast + fixup (robust to trunc or rint):
                # floor = t1 - (t1 > pp); frac = (pp - t1) + (t1 > pp)
                t1 = plp.tile([C, 2, HALFP], i16, tag="t1", name=f"t1_{h3}")
                nc.vector.tensor_copy(out=t1, in_=pp)
                G = plp.tile([C, 2, HALFP], bf16, tag="G", name=f"G{h3}")
                nc.vector.tensor_tensor(out=G, in0=t1, in1=pp, op=ALU.is_gt)
                f4b = plp.tile([C, 2, HALFP], bf16, tag="f4b", name=f"f4{h3}")
                nc.vector.tensor_tensor(out=f4b, in0=pp, in1=t1,
                                        op=ALU.subtract)
                nc.vector.tensor_tensor(out=f4b, in0=f4b, in1=G, op=ALU.add)
                # anchor index iy*WP + ix from the exact integer floor rows
                FA = plp.tile([KT, 2, HALFP], f32, tag="FA", name=f"FA{h3}")
                nc.vector.tensor_tensor(out=FA, in0=t1[0:KT, :, :],
                                        in1=G[0:KT, :, :], op=ALU.subtract)
                A = plp.tile([KT, HALFP], f32, tag="A", name=f"A{h3}")
                nc.vector.scalar_tensor_tensor(
                    out=A, in0=FA[:, 0, :], scalar=float(WP),
                    in1=FA[:, 1, :], op0=ALU.mult, op1=ALU.add)
                idx16 = plp.tile([KT, HALFP], i16, tag="idx16",
                                 name=f"idx16_{h3}")
                nc.vector.tensor_copy(out=idx16, in_=A)

                # corner weight planes (quadrant d = corner d)
                wqh = wq[:, h3 * HALFP: (h3 + 1) * HALFP]
                nc.vector.tensor_tensor(out=wqh, in0=f4b[:, 0, :],
                                        in1=f4b[:, 1, :], op=ALU.mult)
                nc.vector.tensor_tensor(out=wqh[32:41, :],
                                        in0=f4b[32:41, 1, :],
                                        in1=wqh[32:41, :], op=ALU.subtract)
                nc.vector.tensor_tensor(out=wqh[64:73, :],
                                        in0=f4b[64:73, 0, :],
                                        in1=wqh[64:73, :], op=ALU.subtract)
                u = plp.tile([KT, HALFP], bf16, tag="u", name=f"u{h3}")
                nc.vector.tensor_tensor(out=u, in0=f4b[0:KT, 0, :],
                                        in1=f4b[0:KT, 1, :], op=ALU.add)
                nc.vector.scalar_tensor_tensor(
                    out=wqh[0:KT, :], in0=wqh[0:KT, :], scalar=1.0, in1=u,
                    op0=ALU.add, op1=ALU.subtract)

                # stage anchor indices to DRAM (straight copy) and reload
                # 16-wrapped + replicated for the gather ucode
                st = nc.sync.dma_start(out=stgi[h3], in_=idx16[:])
                s_ap = stgi[h3]
                for gg in range(8):
                    ld = nc.sync.dma_start(
                        out=idx_w[gg * 16:(gg + 1) * 16, h3, :, :],
                        in_=bass.AP(s_ap.tensor, s_ap.offset,
                                    [[1, 16], [HALFP, KT], [16, 144]]),
                    )
                    add_dep_helper(ld.ins, st.ins,
                                   reason="idx load after staging write")

                # ---------------- gather + weight + deform conv ------------
                pds = [psD.tile([O, TILE], f32, tag="pd",
                                name=f"pd{h3}_{ci}") for ci in range(NCH)]
                for k in range(KT):
                    Wr4 = wrp.tile([C, HALFP], bf16, tag="wr",
                                   name=f"wr{h3}_{k}")
                    nc.vector.stream_shuffle(
                        out=Wr4[:], in_=wq[:, h3 * HALFP:(h3 + 1) * HALFP],
                        mask=[k] * 32)
                    ags = []
                    for ci in range(NCH):
                        g4 = gp.tile([C, 4, GCH], bf16, tag="g")
                        nc.gpsimd.dma_gather(
                            out_ap=g4[:],
                            in_ap=XT4[:, :],
                            idxs_ap=idx_w[
                                :, h3, k, ci * 48: (ci + 1) * 48],
                            num_idxs=GCH, num_idxs_reg=GCH,
                            elem_size=4 * C, transpose=True,
                            sbuf_tokens_per_rank=128,
                            sbuf_free_dim_per_rank=1024,
                            queue_num=gctr % 4,
                        )
                        gctr += 1
                        # weight in place: g4 *= Wr4 (all-bf16, 2x mode)
                        for chi in range(4):
                            nc.vector.tensor_tensor(
                                out=g4[:, chi, :], in0=g4[:, chi, :],
                                in1=Wr4[:, ci * GCH:(ci + 1) * GCH],
                                op=ALU.mult)
                        ags.append(g4)
                    # weight-stationary matmul order: 4 LDWEIGHTS per tap
                    for chi in range(4):
                        for ci in range(NCH):
                            for c0, cn in ((0, 512), (512, 256)):
                                nc.tensor.matmul(
                                    pds[ci][:, c0: c0 + cn],
                                    wdef4[:, k, chi, :],
                                    ags[ci][:, chi, c0: c0 + cn],
                                    start=(k == 0 and chi == 0),
                                    stop=(k == KT - 1 and chi == 3))
                for ci in range(NCH):
                    col = h3 * HALFP + ci * TILE
                    nc.scalar.activation(
                        out=convout[:, col: col + TILE], in_=pds[ci][:],
                        func=AF.Copy,
                        accum_out=sums[:, h3 * NCH + ci:
                                       h3 * NCH + ci + 1])
                    nc.scalar.activation(
                        out=pds[ci][:], in_=pds[ci][:], func=AF.Square,
                        accum_out=sqsums[:, h3 * NCH + ci:
                                         h3 * NCH + ci + 1])

            if phase == 3:
                nc.sync.dma_start(out=y_d[:, :], in_=convout)

            if phase >= 4:
                # ---------------- BN stats + collective --------------------
                stats = const.tile([O, 2], f32)
                nc.vector.tensor_reduce(out=stats[:, 0:1],
                                        in_=sums[:, 0:2 * NCH],
                                        axis=mybir.AxisListType.X, op=ALU.add)
                nc.vector.tensor_reduce(out=stats[:, 1:2],
                                        in_=sqsums[:, 0:2 * NCH],
                                        axis=mybir.AxisListType.X, op=ALU.add)
                d1 = nc.sync.dma_start(out=ccin[:, :], in_=stats)
                cc = nc.gpsimd.collective_compute(
                    "AllReduce", ALU.add,
                    replica_groups=[list(range(NCORES))],
                    ins=[ccin.ap().opt()], outs=[ccout.ap().opt()],
                )
                add_dep_helper(cc.ins, d1.ins,
                               reason="collective after stats dma")
                gstats = const.tile([O, 2], f32)
                d2 = nc.sync.dma_start(out=gstats, in_=ccout[:, :])
                add_dep_helper(d2.ins, cc.ins,
                               reason="stats load after collective")

                mean = const.tile([O, 1], f32)
                nc.vector.tensor_scalar_mul(out=mean, in0=gstats[:, 0:1],
                                            scalar1=1.0 / NELEM)
                var = const.tile([O, 1], f32)
                nc.vector.tensor_scalar_mul(out=var, in0=gstats[:, 1:2],
                                            scalar1=1.0 / NELEM)
                m2 = const.tile([O, 1], f32)
                nc.vector.tensor_tensor(out=m2, in0=mean, in1=mean,
                                        op=ALU.mult)
                nc.vector.tensor_tensor(out=var, in0=var, in1=m2,
                                        op=ALU.subtract)
                eps = const.tile([O, 1], f32)
                nc.vector.memset(eps, BN_EPS)
                sd = const.tile([O, 1], f32)
                nc.scalar.activation(out=sd, in_=var, func=AF.Sqrt,
                                     bias=eps[:, 0:1])
                rstd = const.tile([O, 1], f32)
                nc.vector.reciprocal(out=rstd, in_=sd)
                scalev = const.tile([O, 1], f32)
                nc.vector.tensor_tensor(out=scalev, in0=rstd, in1=bn[:, 0:1],
                                        op=ALU.mult)
                biasv = const.tile([O, 1], f32)
                nc.vector.tensor_tensor(out=biasv, in0=mean, in1=scalev,
                                        op=ALU.mult)
                nc.vector.tensor_tensor(out=biasv, in0=bn[:, 1:2], in1=biasv,
                                        op=ALU.subtract)
                # BN + ReLU fused, in place (token order; host unpermutes)
                nc.scalar.activation(out=convout[:], in_=convout[:],
                                     func=AF.Relu,
                                     scale=scalev[:, 0:1], bias=biasv[:, 0:1])
                nc.sync.dma_start(out=y_d[:, :], in_=convout)

    nc.compile()
    return nc


def _get_program():
    import os
    phase = int(os.environ.get("KERNEL_PHASE", "4"))
    key = (phase,)
    if key not in _prog_cache:
        _prog_cache[key] = _build_program(phase)
    return _prog_cache[key]


def _host_inputs(x, w_off, b_off, w_def, b_def, gamma, beta):
    """Build the 8 per-core input maps (device compute stays on-device;
    host does layout prep: slabs, grids, weight permutes, quad tokens)."""
    bf = ml_dtypes.bfloat16
    # padded slab per (n, half): rows h0-4 .. h0+52 of the padded image
    xpad = np.zeros((N, C, H + 2 * PAD, WP), np.float32)
    xpad[:, :, PAD: PAD + H, PAD: PAD + W] = x

    # base grids [C, 2, NPOS] in TOKEN order (4 replicas of the 9 taps),
    # b_off folded in
    hl = np.arange(HSH).repeat(W).astype(np.float32)          # [NPOS]
    wgrid = np.tile(np.arange(W), HSH).astype(np.float32)
    p_of_t = _token_perm()
    perm = np.concatenate([p_of_t, HALFP + p_of_t])
    hl_t, wg_t = hl[perm], wgrid[perm]
    tb = np.zeros((4, 32, 2, NPOS), np.float32)
    for k in range(KT):
        ky, kx = k // 3, k % 3
        tb[:, k, 0, :] = hl_t + ky + 3 + b_off[2 * k]
        tb[:, k, 1, :] = wg_t + kx + 3 + b_off[2 * k + 1]
    tb = tb.reshape(128, 2, NPOS)

    woff = np.zeros((C, KT, 128), np.float32)
    for k in range(KT):          # tap index
        ky, kx = k // 3, k % 3
        for r in range(4):
            for yx in range(2):
                for j in range(KT):   # offset-channel tap j -> channel 2j+yx
                    woff[:, k, 32 * r + yx * 9 + j] = \
                        w_off[2 * j + yx, :, ky, kx]

    # deform weights, 4x row-replicated: wdef4[32d + c_lo, k, c_hi, o]
    # = w_def[o, 32*c_hi + c_lo, ky, kx] for every corner quadrant d
    wdef4 = np.zeros((4, 32, KT, 4, O), np.float32)
    for k in range(KT):
        ky, kx = k // 3, k % 3
        wk = w_def[:, :, ky, kx]                    # [O, C]
        for chi in range(4):
            wdef4[:, :, k, chi, :] = wk[:, 32 * chi: 32 * chi + 32].T[None]
    wdef4 = wdef4.reshape(C, KT * 4 * O)

    bn = np.stack([gamma, beta], axis=1).astype(np.float32)

    in_maps = []
    q = np.arange(NTOK)
    for s in range(NCORES):
        n, half = s // 2, s % 2
        slab = np.zeros((C, XLEN), np.float32)
        slab[:, :SLAB] = xpad[n, :, half * HSH: half * HSH + HP, :] \
            .reshape(C, SLAB)
        # quad-token buffer, corner-per-quadrant element order:
        # token q element (c_hi, d, c_lo) = slab[32c_hi + c_lo, q + delta_d]
        slabT = slab.T.astype(np.float32)              # [XLEN, C]
        xt4 = np.zeros((NTOK, 4, 4, 32), np.float32)
        for d, dl in enumerate((0, 1, WP, WP + 1)):
            src = q + dl
            ok = src < XLEN
            xt4[ok, :, d, :] = slabT[src[ok]].reshape(-1, 4, 32)
        # token (r, p) lives at partition p, free els [512r, 512r+512)
        xt4 = xt4.reshape(NTOK // 128, 128, 512).transpose(1, 0, 2) \
                 .reshape(128, 4 * NTOK)
        in_maps.append({
            "xb": slab.astype(bf),
            "xt4": xt4.astype(bf),
            "tb": tb,
            "woff": woff.astype(bf),
            "wdef": wdef4.astype(bf),
            "bn": bn,
        })
    return in_maps


def kernel(x, w_off, b_off, w_def, b_def, gamma, beta):
    x = np.asarray(x, np.float32)
    in_maps = _host_inputs(x, np.asarray(w_off, np.float32),
                           np.asarray(b_off, np.float32),
                           np.asarray(w_def, np.float32),
                           np.asarray(b_def, np.float32),
                           np.asarray(gamma, np.float32),
                           np.asarray(beta, np.float32))
    nc = _get_program()

    import os

    def _run_sim():
        from concourse.bass_interp import MultiCoreSim
        sim = MultiCoreSim(nc, NCORES)
        for s in range(NCORES):
            for k, v in in_maps[s].items():
                sim.cores[s].tensor(k)[:] = v
        sim.simulate()
        return [{"y": np.asarray(sim.cores[s].mem_tensor("y"))}
                for s in range(NCORES)]

    if os.environ.get("KERNEL_SIM"):
        results = _run_sim()
    else:
        try:
            from concourse.bass_utils import run_bass_kernel_spmd
            r = run_bass_kernel_spmd(nc, in_maps, core_ids=list(range(NCORES)))
            results = r.results
        except Exception as e:
            import sys
            print(f"kernel: hardware run failed ({type(e).__name__}: {e}); "
                  f"falling back to CoreSim", file=sys.stderr, flush=True)
            results = _run_sim()

    p_of_t = _token_perm()
    out = np.empty((N, O, H, W), np.float32)
    for s in range(NCORES):
        n, half = s // 2, s % 2
        y = np.asarray(results[s]["y"], dtype=np.float32)   # [O, NPOS] tokens
        ypos = np.empty((O, NPOS), np.float32)
        for h3 in range(2):
            ypos[:, h3 * HALFP + p_of_t] = y[:, h3 * HALFP:(h3 + 1) * HALFP]
        out[n, :, half * HSH: (half + 1) * HSH, :] = \
            ypos.reshape(O, HSH, W)
    return out


# revision 26
# speedup vs baseline: 2.3499x; 2.3499x over previous
"""DeformConv2dBlock (offset-conv -> deformable 3x3 conv -> train-mode BN -> ReLU)
as a Bass/Tile SPMD kernel on 8 TRN2 NeuronCores.

Sharding: data-parallel over (batch n, image half): core s handles
n = s//2, rows h0 = (s%2)*48 .. h0+48.  Params replicated.  BN batch stats
get an 8-core AllReduce for exact training-mode parity.

v3 pipeline (per core) -- replication-matmul-free:
  1. Host pre-builds XT4, a position-major QUAD-token buffer with
     corner-per-quadrant element order: token q element (c_hi, d, c_lo)
     = slab[32*c_hi + c_lo, q + delta_d], delta = (0, 1, WP, WP+1).
     After the transposing gather, corner d's data lands on partition
     quadrant d: g4[32*d + c_lo, c_hi, tok].
  2. PE: offset conv (9 shifted matmuls, bf16).  DVE/ACT: sampling
     positions, frac via f32 mod, bilinear corner weight planes wq
     (bf16, quadrant d rows 32d+k = corner-d weight of tap k), int16
     anchors.  Everything runs in gather-token order (host permutes the
     base grid), so no strided engine reads anywhere in the main loop.
  3. stream_shuffle (DVE 32x32 reorder, mask=[k]*32) broadcasts tap k's
     corner weights across each quadrant: ONE op replaces the per-corner
     K=9 replication matmuls of v2.
  4. Q7: ONE dma_gather per (half, tap, 768-chunk), rotated over 4 SWDGE
     queues so transfers overlap descriptor-gen.
  5. DVE: ag4 = g4 * Wr4 (all-bf16, 2x mode).  PE: deform conv via
     4x-row-replicated weights wdef4[k, c_hi]: psum[o,t] += sum_{32d+c_lo}
     wdef[32c_hi+c_lo, o] * w_d(t) * x[32c_hi+c_lo, anchor_t + delta_d],
     accumulated over (k, c_hi) -- 36 matmul-pairs per 768-token tile.
  6. ACT: per-channel sum/sumsq, AllReduce over 8 cores, in-place BN+ReLU
     on bf16 convout, host unpermutes token->position order.
"""

import numpy as np
import ml_dtypes

# ---------------------------------------------------------------- constants
N, C, O, H, W = 4, 128, 128, 96, 96
KT = 9                      # 3x3 taps
PAD = 4
HSH = H // 2                # 48 rows per shard
HP = HSH + 2 * PAD          # 56 slab rows
WP = W + 2 * PAD            # 104 slab cols
SLAB = HP * WP              # 5824
XLEN = 6160                 # padded slab row length (>= SLAB + WP + 2, /16)
NTOK = 6144                 # quad tokens in XT4 (48 ranks * 128)
NPOS = HSH * W              # 4608 output positions per core
HALFP = NPOS // 2           # 2304
GCH = 768                   # gather chunk (descriptor-ring limit < 1024)
NCH = HALFP // GCH          # 3
TILE = GCH                  # psum col tile
NCORES = 8
BN_EPS = 1e-5
NELEM = N * H * W           # BN normalizer 36864

_prog_cache = {}


def _token_perm():
    """token t (within a 2304-position half) -> position p'.

    The device stores position p' at token t = 144*(p'%16) + p'//16 (the
    gather's 16-partition wrap); the inverse is p' = 16*(t%144) + t//144.
    """
    t = np.arange(HALFP)
    return 16 * (t % 144) + t // 144


def _build_program(phase=4):
    import concourse.bass as bass
    import concourse.bacc as bacc
    import concourse.tile as tile
    import concourse.mybir as mybir

    f32 = mybir.dt.float32
    bf16 = mybir.dt.bfloat16
    i16 = mybir.dt.int16
    AF = mybir.ActivationFunctionType
    ALU = mybir.AluOpType

    nc = bacc.Bacc("TRN2", target_bir_lowering=False, num_devices=NCORES,
                   num_swdge_queues=4)

    xb_d = nc.declare_dram_parameter("xb", [C, XLEN], bf16, isOutput=False)
    xt4_d = nc.declare_dram_parameter("xt4", [C, 4 * NTOK], bf16, isOutput=False)
    tb_d = nc.declare_dram_parameter("tb", [C, 2, NPOS], f32, isOutput=False)
    woff_d = nc.declare_dram_parameter("woff", [C, KT, 128], bf16, isOutput=False)
    wdef_d = nc.declare_dram_parameter("wdef", [C, KT * 4 * O], bf16,
                                       isOutput=False)
    bn_d = nc.declare_dram_parameter("bn", [O, 2], f32, isOutput=False)
    y_d = nc.declare_dram_parameter("y", [O, NPOS], bf16, isOutput=True)

    ccin = nc.dram_tensor("ccin", [O, 2], f32)
    ccout = nc.dram_tensor("ccout", [O, 2], f32)
    # anchor-index staging in DRAM (straight copy; reload does the 16-wrap)
    stgi = nc.dram_tensor("stgi", [2, KT, HALFP], i16)

    def vap(t, off_el, dims):
        """Raw AP view of tile t at element offset off_el with free dims."""
        a = t[:]
        return bass.AP(a.tensor, a.offset + off_el, [a.ap[0]] + dims)

    from concourse.tile import add_dep_helper

    with tile.TileContext(nc) as tc:
        import contextlib
        est = contextlib.ExitStack()
        with est:
            const = est.enter_context(tc.tile_pool(name="const", bufs=1))

            woff = const.tile([C, KT, 128], bf16)
            nc.sync.dma_start(out=woff, in_=woff_d[:, :, :])
            wdef4 = const.tile([C, KT, 4, O], bf16)
            nc.sync.dma_start(out=wdef4[:], in_=wdef_d[:, :])
            bn = const.tile([O, 2], f32)
            nc.sync.dma_start(out=bn, in_=bn_d[:, :])

            XT4 = const.tile([C, 4 * NTOK], bf16)      # quad-token buffer
            idx_w = const.tile([C, 2, KT, 144], i16)   # 16-wrapped anchor idx
            # corner weight planes, token order, bf16.  Row 32d + k holds
            # corner d's weight for tap k (replica rows 32d+9..32d+31 unused).
            wq = const.tile([C, NPOS], bf16)
            # offsets in token order: row 32r + j = tap j (replica r),
            # free dims (yx, token)
            off_h = const.tile([C, 2, HALFP], bf16)
            nc.vector.memset(off_h, 0.0)

            convout = const.tile([O, NPOS], bf16)          # pre-BN conv out
            sums = const.tile([O, 2 * NCH], f32)
            sqsums = const.tile([O, 2 * NCH], f32)

            xbp = est.enter_context(tc.tile_pool(name="xbp", bufs=1))
            psA = est.enter_context(
                tc.tile_pool(name="psA", bufs=2, space="PSUM"))
            plp = est.enter_context(tc.tile_pool(name="plp", bufs=1))
            wrp = est.enter_context(tc.tile_pool(name="wrp", bufs=2))
            gp = est.enter_context(tc.tile_pool(name="gp", bufs=3))
            psD = est.enter_context(
                tc.tile_pool(name="psD", bufs=3, space="PSUM"))

            xb = xbp.tile([C, XLEN], bf16)
            nc.sync.dma_start(out=xb, in_=xb_d[:, :])
            nc.sync.dma_start(out=XT4, in_=xt4_d[:, :])

            gctr = 0
            for h3 in range(2):
                # ---------------- offset conv (position order in PSUM) -----
                for t in range(6):
                    T = h3 * 6 + t
                    po = psA.tile([C, 384], f32, tag="poff", name=f"po{T}")
                    for k in range(KT):
                        ky, kx = k // 3, k % 3
                        rhs = vap(xb, (4 * T + ky + 3) * WP + kx + 3,
                                  [[WP, 4], [1, W]])
                        nc.tensor.matmul(po[:], woff[:, k, :], rhs,
                                         start=(k == 0), stop=(k == KT - 1))
                    # contiguous position-order redistribution (fast DMAs);
                    # the token permute is applied by the pp add's in1 AP
                    ostg = xbp.tile([C, 384], bf16, tag="ostg",
                                    name=f"ostg{T}")
                    nc.scalar.activation(out=ostg, in_=po[:], func=AF.Copy)
                    for r in range(4):
                        for yx in range(2):
                            nc.sync.dma_start(
                                out=off_h[32 * r: 32 * r + 9, yx,
                                          t * 384: (t + 1) * 384],
                                in_=ostg[32 * r + 9 * yx:
                                         32 * r + 9 * yx + 9, :],
                            )

                # ---------------- positions (token order) ------------------
                pp = plp.tile([C, 2, HALFP], f32, tag="pp", name=f"pp{h3}")
                nc.sync.dma_start(
                    out=pp, in_=tb_d[:, :, h3 * HALFP: (h3 + 1) * HALFP])
                # pp[token t] += off_h[position 16*(t%144) + t//144]
                # (split per yx plane: DVE tensor_tensor APs max out at 3D)
                ppa = pp[:]
                oha = off_h[:]
                for yx in range(2):
                    ppv = bass.AP(ppa.tensor, ppa.offset + yx * HALFP,
                                  [ppa.ap[0], [144, 16], [1, 144]])
                    ohv = bass.AP(oha.tensor, oha.offset + yx * HALFP,
                                  [oha.ap[0], [1, 16], [16, 144]])
                    nc.vector.tensor_tensor(out=ppv, in0=ppv, in1=ohv,
                                            op=ALU.add)
                # clamp sampling coords into the zero-padded slab
                nc.vector.tensor_scalar(out=pp[:, 0, :], in0=pp[:, 0, :],
                                        scalar1=0.01,
                                        scalar2=float(HP - 1.1),
                                        op0=ALU.max, op1=ALU.min)
                nc.vector.tensor_scalar(out=pp[:, 1, :], in0=pp[:, 1, :],
                                        scalar1=0.01,
                                        scalar2=float(WP - 1.1),
                                        op0=ALU.max, op1=ALU.min)
                # floor via int cast + fixup (robust to trunc or rint):
                # floor = t1 - (t1 > pp); frac = (pp - t1) + (t1 > pp)
                t1 = plp.tile([C, 2, HALFP], i16, tag="t1", name=f"t1_{h3}")
                nc.vector.tensor_copy(out=t1, in_=pp)
                G = plp.tile([C, 2, HALFP], bf16, tag="G", name=f"G{h3}")
                nc.vector.tensor_tensor(out=G, in0=t1, in1=pp, op=ALU.is_gt)
                f4b = plp.tile([C, 2, HALFP], bf16, tag="f4b", name=f"f4{h3}")
                nc.vector.tensor_tensor(out=f4b, in0=pp, in1=t1,
                                        op=ALU.subtract)
                nc.vector.tensor_tensor(out=f4b, in0=f4b, in1=G, op=ALU.add)
                # anchor index iy*WP + ix from the exact integer floor rows
                FA = plp.tile([KT, 2, HALFP], f32, tag="FA", name=f"FA{h3}")
                nc.vector.tensor_tensor(out=FA, in0=t1[0:KT, :, :],
                                        in1=G[0:KT, :, :], op=ALU.subtract)
                nc.vector.scalar_tensor_tensor(
                    out=FA[:, 0, :], in0=FA[:, 0, :], scalar=float(WP),
                    in1=FA[:, 1, :], op0=ALU.mult, op1=ALU.add)
                # cast + 16x144 free-dim transpose in one op, so the DRAM
                # staging and the wrapped reload below are contiguous-run
                # DMAs: idx16[k, 144q + c] = anchor(token 16c + q)
                idx16 = plp.tile([KT, HALFP], i16, tag="idx16",
                                 name=f"idx16_{h3}")
                wa = idx16[:]
                fa0 = FA[:, 0, :]
                nc.vector.tensor_copy(
                    out=bass.AP(wa.tensor, wa.offset,
                                [wa.ap[0], [144, 16], [1, 144]]),
                    in_=bass.AP(fa0.tensor, fa0.offset,
                                [fa0.ap[0], [1, 16], [16, 144]]))

                # corner weight planes (quadrant d = corner d)
                wqh = wq[:, h3 * HALFP: (h3 + 1) * HALFP]
                nc.vector.tensor_tensor(out=wqh, in0=f4b[:, 0, :],
                                        in1=f4b[:, 1, :], op=ALU.mult)
                nc.vector.tensor_tensor(out=wqh[32:41, :],
                                        in0=f4b[32:41, 1, :],
                                        in1=wqh[32:41, :], op=ALU.subtract)
                nc.vector.tensor_tensor(out=wqh[64:73, :],
                                        in0=f4b[64:73, 0, :],
                                        in1=wqh[64:73, :], op=ALU.subtract)
                u = plp.tile([KT, HALFP], bf16, tag="u", name=f"u{h3}")
                nc.vector.tensor_tensor(out=u, in0=f4b[0:KT, 0, :],
                                        in1=f4b[0:KT, 1, :], op=ALU.add)
                nc.vector.scalar_tensor_tensor(
                    out=wqh[0:KT, :], in0=wqh[0:KT, :], scalar=1.0, in1=u,
                    op0=ALU.add, op1=ALU.subtract)

                # stage anchor indices to DRAM (straight copy) and reload
                # 16-wrapped + replicated for the gather ucode
                st = nc.sync.dma_start(out=stgi[h3], in_=idx16[:])
                s_ap = stgi[h3]
                for gg in range(8):
                    ld = nc.sync.dma_start(
                        out=idx_w[gg * 16:(gg + 1) * 16, h3, :, :],
                        in_=bass.AP(s_ap.tensor, s_ap.offset,
                                    [[144, 16], [HALFP, KT], [1, 144]]),
                    )
                    add_dep_helper(ld.ins, st.ins,
                                   reason="idx load after staging write")

                # ---------------- gather + weight + deform conv ------------
                pds = [psD.tile([O, TILE], f32, tag="pd",
                                name=f"pd{h3}_{ci}") for ci in range(NCH)]
                for k in range(KT):
                    Wr4 = wrp.tile([C, HALFP], bf16, tag="wr",
                                   name=f"wr{h3}_{k}")
                    nc.vector.stream_shuffle(
                        out=Wr4[:], in_=wq[:, h3 * HALFP:(h3 + 1) * HALFP],
                        mask=[k] * 32)
                    ags = []
                    for ci in range(NCH):
                        g4 = gp.tile([C, 4, GCH], bf16, tag="g")
                        nc.gpsimd.dma_gather(
                            out_ap=g4[:],
                            in_ap=XT4[:, :],
                            idxs_ap=idx_w[
                                :, h3, k, ci * 48: (ci + 1) * 48],
                            num_idxs=GCH, num_idxs_reg=GCH,
                            elem_size=4 * C, transpose=True,
                            sbuf_tokens_per_rank=128,
                            sbuf_free_dim_per_rank=1024,
                            queue_num=gctr % 4,
                        )
                        gctr += 1
                        # weight in place: g4 *= Wr4 (all-bf16, 2x mode)
                        for chi in range(4):
                            nc.vector.tensor_tensor(
                                out=g4[:, chi, :], in0=g4[:, chi, :],
                                in1=Wr4[:, ci * GCH:(ci + 1) * GCH],
                                op=ALU.mult)
                        ags.append(g4)
                    # weight-stationary matmul order: 4 LDWEIGHTS per tap
                    for chi in range(4):
                        for ci in range(NCH):
                            for c0, cn in ((0, 512), (512, 256)):
                                nc.tensor.matmul(
                                    pds[ci][:, c0: c0 + cn],
                                    wdef4[:, k, chi, :],
                                    ags[ci][:, chi, c0: c0 + cn],
                                    start=(k == 0 and chi == 0),
                                    stop=(k == KT - 1 and chi == 3))
                for ci in range(NCH):
                    col = h3 * HALFP + ci * TILE
                    nc.scalar.activation(
                        out=convout[:, col: col + TILE], in_=pds[ci][:],
                        func=AF.Copy,
                        accum_out=sums[:, h3 * NCH + ci:
                                       h3 * NCH + ci + 1])
                    nc.scalar.activation(
                        out=pds[ci][:], in_=pds[ci][:], func=AF.Square,
                        accum_out=sqsums[:, h3 * NCH + ci:
                                         h3 * NCH + ci + 1])

            if phase == 3:
                nc.sync.dma_start(out=y_d[:, :], in_=convout)

            if phase >= 4:
                # ---------------- BN stats + collective --------------------
                stats = const.tile([O, 2], f32)
                nc.vector.tensor_reduce(out=stats[:, 0:1],
                                        in_=sums[:, 0:2 * NCH],
                                        axis=mybir.AxisListType.X, op=ALU.add)
                nc.vector.tensor_reduce(out=stats[:, 1:2],
                                        in_=sqsums[:, 0:2 * NCH],
                                        axis=mybir.AxisListType.X, op=ALU.add)
                d1 = nc.sync.dma_start(out=ccin[:, :], in_=stats)
                cc = nc.gpsimd.collective_compute(
                    "AllReduce", ALU.add,
                    replica_groups=[list(range(NCORES))],
                    ins=[ccin.ap().opt()], outs=[ccout.ap().opt()],
                )
                add_dep_helper(cc.ins, d1.ins,
                               reason="collective after stats dma")
                gstats = const.tile([O, 2], f32)
                d2 = nc.sync.dma_start(out=gstats, in_=ccout[:, :])
                add_dep_helper(d2.ins, cc.ins,
                               reason="stats load after collective")

                mean = const.tile([O, 1], f32)
                nc.vector.tensor_scalar_mul(out=mean, in0=gstats[:, 0:1],
                                            scalar1=1.0 / NELEM)
                var = const.tile([O, 1], f32)
                nc.vector.tensor_scalar_mul(out=var, in0=gstats[:, 1:2],
                                            scalar1=1.0 / NELEM)
                m2 = const.tile([O, 1], f32)
                nc.vector.tensor_tensor(out=m2, in0=mean, in1=mean,
                                        op=ALU.mult)
                nc.vector.tensor_tensor(out=var, in0=var, in1=m2,
                                        op=ALU.subtract)
                eps = const.tile([O, 1], f32)
                nc.vector.memset(eps, BN_EPS)
                sd = const.tile([O, 1], f32)
                nc.scalar.activation(out=sd, in_=var, func=AF.Sqrt,
                                     bias=eps[:, 0:1])
                rstd = const.tile([O, 1], f32)
                nc.vector.reciprocal(out=rstd, in_=sd)
                scalev = const.tile([O, 1], f32)
                nc.vector.tensor_tensor(out=scalev, in0=rstd, in1=bn[:, 0:1],
                                        op=ALU.mult)
                biasv = const.tile([O, 1], f32)
                nc.vector.tensor_tensor(out=biasv, in0=mean, in1=scalev,
                                        op=ALU.mult)
                nc.vector.tensor_tensor(out=biasv, in0=bn[:, 1:2], in1=biasv,
                                        op=ALU.subtract)
                # BN + ReLU fused, in place (token order; host unpermutes)
                nc.scalar.activation(out=convout[:], in_=convout[:],
                                     func=AF.Relu,
                                     scale=scalev[:, 0:1], bias=biasv[:, 0:1])
                nc.sync.dma_start(out=y_d[:, :], in_=convout)

    nc.compile()
    return nc


def _get_program():
    import os
    phase = int(os.environ.get("KERNEL_PHASE", "4"))
    key = (phase,)
    if key not in _prog_cache:
        _prog_cache[key] = _build_program(phase)
    return _prog_cache[key]


def _host_inputs(x, w_off, b_off, w_def, b_def, gamma, beta):
    """Build the 8 per-core input maps (device compute stays on-device;
    host does layout prep: slabs, grids, weight permutes, quad tokens)."""
    bf = ml_dtypes.bfloat16
    # padded slab per (n, half): rows h0-4 .. h0+52 of the padded image
    xpad = np.zeros((N, C, H + 2 * PAD, WP), np.float32)
    xpad[:, :, PAD: PAD + H, PAD: PAD + W] = x

    # base grids [C, 2, NPOS] in TOKEN order (4 replicas of the 9 taps),
    # b_off folded in
    hl = np.arange(HSH).repeat(W).astype(np.float32)          # [NPOS]
    wgrid = np.tile(np.arange(W), HSH).astype(np.float32)
    p_of_t = _token_perm()
    perm = np.concatenate([p_of_t, HALFP + p_of_t])
    hl_t, wg_t = hl[perm], wgrid[perm]
    tb = np.zeros((4, 32, 2, NPOS), np.float32)
    for k in range(KT):
        ky, kx = k // 3, k % 3
        tb[:, k, 0, :] = hl_t + ky + 3 + b_off[2 * k]
        tb[:, k, 1, :] = wg_t + kx + 3 + b_off[2 * k + 1]
    tb = tb.reshape(128, 2, NPOS)

    woff = np.zeros((C, KT, 128), np.float32)
    for k in range(KT):          # tap index
        ky, kx = k // 3, k % 3
        for r in range(4):
            for yx in range(2):
                for j in range(KT):   # offset-channel tap j -> channel 2j+yx
                    woff[:, k, 32 * r + yx * 9 + j] = \
                        w_off[2 * j + yx, :, ky, kx]

    # deform weights, 4x row-replicated: wdef4[32d + c_lo, k, c_hi, o]
    # = w_def[o, 32*c_hi + c_lo, ky, kx] for every corner quadrant d
    wdef4 = np.zeros((4, 32, KT, 4, O), np.float32)
    for k in range(KT):
        ky, kx = k // 3, k % 3
        wk = w_def[:, :, ky, kx]                    # [O, C]
        for chi in range(4):
            wdef4[:, :, k, chi, :] = wk[:, 32 * chi: 32 * chi + 32].T[None]
    wdef4 = wdef4.reshape(C, KT * 4 * O)

    bn = np.stack([gamma, beta], axis=1).astype(np.float32)

    in_maps = []
    q = np.arange(NTOK)
    for s in range(NCORES):
        n, half = s // 2, s % 2
        slab = np.zeros((C, XLEN), np.float32)
        slab[:, :SLAB] = xpad[n, :, half * HSH: half * HSH + HP, :] \
            .reshape(C, SLAB)
        # quad-token buffer, corner-per-quadrant element order:
        # token q element (c_hi, d, c_lo) = slab[32c_hi + c_lo, q + delta_d]
        slabT = slab.T.astype(np.float32)              # [XLEN, C]
        xt4 = np.zeros((NTOK, 4, 4, 32), np.float32)
        for d, dl in enumerate((0, 1, WP, WP + 1)):
            src = q + dl
            ok = src < XLEN
            xt4[ok, :, d, :] = slabT[src[ok]].reshape(-1, 4, 32)
        # token (r, p) lives at partition p, free els [512r, 512r+512)
        xt4 = xt4.reshape(NTOK // 128, 128, 512).transpose(1, 0, 2) \
                 .reshape(128, 4 * NTOK)
        in_maps.append({
            "xb": slab.astype(bf),
            "xt4": xt4.astype(bf),
            "tb": tb,
            "woff": woff.astype(bf),
            "wdef": wdef4.astype(bf),
            "bn": bn,
        })
    return in_maps


def kernel(x, w_off, b_off, w_def, b_def, gamma, beta):
    x = np.asarray(x, np.float32)
    in_maps = _host_inputs(x, np.asarray(w_off, np.float32),
                           np.asarray(b_off, np.float32),
                           np.asarray(w_def, np.float32),
                           np.asarray(b_def, np.float32),
                           np.asarray(gamma, np.float32),
                           np.asarray(beta, np.float32))
    nc = _get_program()

    import os

    def _run_sim():
        from concourse.bass_interp import MultiCoreSim
        sim = MultiCoreSim(nc, NCORES)
        for s in range(NCORES):
            for k, v in in_maps[s].items():
                sim.cores[s].tensor(k)[:] = v
        sim.simulate()
        return [{"y": np.asarray(sim.cores[s].mem_tensor("y"))}
                for s in range(NCORES)]

    if os.environ.get("KERNEL_SIM"):
        results = _run_sim()
    else:
        try:
            from concourse.bass_utils import run_bass_kernel_spmd
            r = run_bass_kernel_spmd(nc, in_maps, core_ids=list(range(NCORES)))
            results = r.results
        except Exception as e:
            import sys
            print(f"kernel: hardware run failed ({type(e).__name__}: {e}); "
                  f"falling back to CoreSim", file=sys.stderr, flush=True)
            results = _run_sim()

    p_of_t = _token_perm()
    out = np.empty((N, O, H, W), np.float32)
    for s in range(NCORES):
        n, half = s // 2, s % 2
        y = np.asarray(results[s]["y"], dtype=np.float32)   # [O, NPOS] tokens
        ypos = np.empty((O, NPOS), np.float32)
        for h3 in range(2):
            ypos[:, h3 * HALFP + p_of_t] = y[:, h3 * HALFP:(h3 + 1) * HALFP]
        out[n, :, half * HSH: (half + 1) * HSH, :] = \
            ypos.reshape(O, HSH, W)
    return out
